# revision 45
# baseline (speedup 1.0000x reference)
"""Trainium2 Bass kernel for CubeFaceNN.

Computes, for x of shape [8, 1, 128, 128, 128] (f32):
    out[b, i, p] = relu(x[b, 0, p] - x[b, 0, p + OFF[i]])   (zero padded)
with OFF = [(0,-1,-1), (-1,0,-1), (1,-1,-1), (-1,1,-1), (-1,-1,0), (-1,-1,1)]
(derived from the reference's adj % 3 - 1 indexing).

Sharding: pure data parallel — batch b -> NeuronCore b (8 cores).

Layout: depth d on the 128 SBUF partitions, (h, w) in the free dims.
Pipeline per core:
  1. x (f32) -> SBUF via 4x 2 MiB HWDGE DMAs on the two rings (nc.sync /
     nc.scalar) — ~400 GB/s combined for ready-to-run full-partition
     loads, keeping SWDGE + the GpSimd Q7 cores free for stores.
  2. ACT casts x -> xt16 (fp16, chunked). All compute is fp16: the 2e-2
     error gate admits fp16 input rounding (max-norm rel err ~6e-4) and
     fp16 runs DVE tensor ops ~1.5-2x faster and the PE shift at column
     rate.
  3. PE builds the partition-shifted copy xp16[d] = xt16[d+1] with a
     one-subdiagonal shift matrix (one-hot rows -> values exact);
     ACT copies PSUM -> xp16, interleaved into the unit waves.
  4. 24 h-quarter units (6 channels x 4 waves, 8 och buffers): DVE subs
     on flat contiguous APs; relu on DVE (ch0/2) or ACT (ch1/3/4/5);
     boundary strips relu(A) patch rows/cols where the shifted source is
     zero padding; od != 0 channels use the substituted frame
     out[i, d'+1] = relu(xp16[d'] - xt16[d', h+oh, w+ow]).
  5. Stores (fp16): ch0 and ch2 units are full-128-partition (ch2's
     missing d=127 output plane relu(x[127]) is computed INTO partition
     127 of its och tile) and ride the HWDGE rings; od=-1 channels
     (127 partitions) go SWDGE as half-partition pairs with 8 KiB
     descriptors — a 127-partition ring DMA degenerates to serial
     single-engine descriptor processing (trace-verified ~8x slower).
  The four od=-1 d=0 planes out[i, 0] = relu(x[0]) are stored from one
  [h, w]-layout tile on the rings.
"""

import numpy as np

import concourse.bacc as bacc
import concourse.mybir as mybir
import concourse.tile as tile
from concourse.bass_utils import run_bass_kernel_spmd

D = H = W = 128
HW = H * W
HALF = 64
UH = 32  # unit = h-quarter
UF = UH * W
NU = H // UH
N_CORES = 8
MMF = 512  # matmul moving free size (one PSUM bank of f32)
NCHUNK = HW // MMF
F32 = mybir.dt.float32
F16 = mybir.dt.float16

# (od, oh, ow) per output channel
OFFSETS = [(0, -1, -1), (-1, 0, -1), (1, -1, -1), (-1, 1, -1), (-1, -1, 0), (-1, -1, 1)]
DVE_RELU = (0, 2, 4, 5)  # interior relus on DVE (fp16 tensor_scalar ~4x)
DVE_STRIP = (0, 2)  # strided strips stay mostly on ACT

_NC_CACHE = {}


def build_nc(debug=False):
    nc = bacc.Bacc("TRN2", target_bir_lowering=False, debug=debug)
    x = nc.dram_tensor("x", [D, H, W], F32, kind="ExternalInput")
    out = nc.dram_tensor("out", [6, D, H, W], F16, kind="ExternalOutput")
    # shift matrix: sh[k, m] = 1 iff k == m-1, so (sh.T @ v)[m] = v[m-1]
    # (column 0 is all-zero -> xm16[0] = 0, the zero padding at d = -1)
    sh_dram = nc.inline_tensor(np.eye(D, k=1, dtype=np.float16), name="shift")

    sub = mybir.AluOpType.subtract
    relu = mybir.ActivationFunctionType.Relu
    rings = [nc.sync, nc.scalar]

    with tile.TileContext(nc) as tc:
        with (
            tc.tile_pool(name="xt32", bufs=1) as xt32_pool,
            tc.tile_pool(name="xt16", bufs=1) as xt16_pool,
            tc.tile_pool(name="xm16", bufs=1) as xm16_pool,
            tc.tile_pool(name="sh", bufs=1) as sh_pool,
            tc.tile_pool(name="och", bufs=9) as och_pool,
            tc.tile_pool(name="pf32", bufs=1) as pf32_pool,
            tc.tile_pool(name="pf16", bufs=1) as pf16_pool,
            tc.tile_pool(name="ps", bufs=8, space="PSUM") as ps_pool,
        ):
            sht = sh_pool.tile([D, D], F16)
            nc.sync.dma_start(out=sht[:], in_=sh_dram[:])

            # x -> SBUF f32: 2 MiB full-partition chunks. Rows 0-95 go as
            # HWDGE DMAs alternating the two rings; the last-needed chunk
            # (rows 96-127) rides the otherwise-idle SWDGE path in
            # parallel, shortening the load phase.
            xt32 = xt32_pool.tile([D, H, W], F32)
            for c in range(3):
                hsl = slice(c * 32, (c + 1) * 32)
                rings[c % 2].dma_start(out=xt32[:, hsl], in_=x[:, hsl])
            nc.gpsimd.dma_start(out=xt32[:, 96:128], in_=x[:, 96:128])
            xt32_2 = xt32.rearrange("d h w -> d (h w)")

            # fp16 working copy (ACT cast, chunked so PE + subs start early;
            # later chunks are interleaved into the waves below)
            xt16 = xt16_pool.tile([D, H, W], F16)
            xt2 = xt16.rearrange("d h w -> d (h w)")
            CAST = HW // 8
            cast_next = 0

            def emit_casts(n):
                nonlocal cast_next
                for j in range(cast_next, min(8, cast_next + n)):
                    fsl = slice(j * CAST, (j + 1) * CAST)
                    nc.scalar.copy(out=xt2[:, fsl], in_=xt32_2[:, fsl])
                cast_next = min(8, cast_next + n)

            emit_casts(3)  # rows 0-47: what wave 0's xt-side subs read

            # xm16[d] = xt16[d-1] (xm16[0] = 0) via PE shift matmul (fp16
            # one-hot, exact). With the DOWN-shift, the od=-1 channels run
            # in the DIRECT frame out[i, d] = relu(xt16[d] - xm16[d, ...])
            # on all 128 partitions — the d=0 plane relu(x[0]) falls out of
            # xm16[0] = 0 — so their stores are full-partition ring DMAs.
            # Emission cadence per 512-f16 chunk c: cast (ACT, once per 4
            # chunks) -> matmul (PE) -> PSUM copy (ACT). Program order IS
            # Tile's hazard order, so every producer must be emitted
            # before its consumers.
            xm16 = xm16_pool.tile([D, H, W], F16)
            xm2 = xm16.rearrange("d h w -> d (h w)")
            copy_next = 0

            def emit_copies(n):
                nonlocal copy_next
                for c in range(copy_next, min(NCHUNK, copy_next + n)):
                    emit_casts((c * MMF + MMF - 1) // CAST + 1 - cast_next)
                    ps = ps_pool.tile([D, MMF], F32)
                    nc.tensor.matmul(
                        out=ps[:],
                        lhsT=sht[:],
                        rhs=xt2[:, c * MMF : (c + 1) * MMF],
                        start=True,
                        stop=True,
                    )
                    nc.scalar.copy(out=xm2[:, c * MMF : (c + 1) * MMF], in_=ps[:])
                copy_next = min(NCHUNK, copy_next + n)

            def emit_compute(i, u, och):
                od, oh, ow = OFFSETS[i]
                delta = oh * W + ow
                # A = operand aligned with the output partition frame
                # (strips read it); S = the shifted operand. ch2 runs in
                # the substituted frame och[d'] = out[2, d'-1] =
                # relu(xm16[d'] - xt16[d', h+oh, w+ow]); partition 0 is
                # dead and not stored.
                A3 = xm16 if od == 1 else xt16
                A2 = xm2 if od == 1 else xt2
                S2 = xt2 if od == 1 else (xm2 if od == -1 else xt2)

                hs, he = max(0, -oh), H - max(0, oh)
                f0, f1 = u * UF, (u + 1) * UF
                lo = max(f0, -delta)
                hi = min(f1, HW - delta)
                on_dve = i in DVE_RELU

                och2 = och.rearrange("d h w -> d (h w)")
                r0 = u * UH
                nc.vector.tensor_tensor(
                    out=och2[:, lo - f0 : hi - f0],
                    in0=A2[:, lo:hi],
                    in1=S2[:, lo + delta : hi + delta],
                    op=sub,
                )

                # strips: shifted source is zero-padding there -> relu(A)
                def strip(osel, asel):
                    if i in DVE_STRIP:
                        nc.vector.tensor_scalar_max(och[osel], A3[asel], 0.0)
                    else:
                        nc.scalar.activation(och[osel], A3[asel], relu)

                if oh == -1 and u == 0:
                    strip((slice(0, D), slice(0, 1)), (slice(0, D), slice(0, 1)))
                if oh == 1 and u == NU - 1:
                    strip(
                        (slice(0, D), slice(UH - 1, UH)),
                        (slice(0, D), slice(H - 1, H)),
                    )
                if ow != 0:
                    wb = 0 if ow == -1 else W - 1
                    rs, re = max(hs, r0), min(he, r0 + UH)
                    strip(
                        (slice(0, D), slice(rs - r0, re - r0), slice(wb, wb + 1)),
                        (slice(0, D), slice(rs, re), slice(wb, wb + 1)),
                    )
                # interior relu (in place, fp16)
                osel = och2[:, lo - f0 : hi - f0]
                if on_dve:
                    nc.vector.tensor_scalar_max(osel, osel, 0.0)
                else:
                    nc.scalar.activation(osel, osel, relu)

            unit_no = 0

            def emit_store(i, u, och):
                r0 = u * UH
                if i == 2:
                    # substituted frame, 127 live planes: the first 64
                    # partitions ride the sync ring (64-partition ring
                    # DMAs spread fine; only non-power-of-two widths
                    # degenerate), the 63-partition rest goes SWDGE
                    nc.sync.dma_start(
                        out=out[2, 0:HALF, r0 : r0 + UH], in_=och[1 : 1 + HALF]
                    )
                    nc.gpsimd.dma_start(
                        out=out[2, HALF : D - 1, r0 : r0 + UH],
                        in_=och[1 + HALF : D],
                    )
                else:
                    # full-128-partition ring DMA, emitted right after the
                    # unit's relu. DVE-relu'd units trigger via the sync
                    # ring (SP runs no compute, so a waiting trigger blocks
                    # nothing); ACT-relu'd units via the scalar ring, where
                    # the trigger directly follows its relu in ACT order.
                    ring = nc.sync if i in DVE_RELU else nc.scalar
                    ring.dma_start(out=out[i, :, r0 : r0 + UH], in_=och[:])

            # 4 waves of h-quarters; xp copies interleaved into ACT's
            # stream: the 9 chunks wave 0 reads come right after ch0's
            # unit, later waves' chunks trickle at each wave tail
            SUB_ORDER = (0, 2, 1, 3, 4, 5)  # shift-free channel first
            for u in range(NU):
                for j, i in enumerate(SUB_ORDER):
                    och = och_pool.tile([D, UH, W], F16, name="och")
                    emit_compute(i, u, och)
                    emit_store(i, u, och)
                    if j == 0:
                        emit_copies(9 if u == 0 else 4)
                    elif j >= 4:
                        emit_copies(2)
                if u == 0:
                    # ch2's d=127 boundary plane out[2, 127] = relu(x[127])
                    # (h on partitions) — emitted after wave 0 so its load
                    # never heads off the ring FIFO or the ACT queue during
                    # the ramp
                    p1s = pf32_pool.tile([H, W], F32)
                    p1 = pf16_pool.tile([H, W], F16)
                    nc.scalar.dma_start(out=p1s[:], in_=x[D - 1])
                    nc.scalar.activation(p1[:], p1s[:], relu)
                    nc.scalar.dma_start(out=out[2, D - 1], in_=p1[:])

    nc.compile()
    return nc


def _get_nc():
    if "nc" not in _NC_CACHE:
        _NC_CACHE["nc"] = build_nc()
    return _NC_CACHE["nc"]


def kernel(x: np.ndarray) -> np.ndarray:
    assert x.shape == (N_CORES, 1, D, H, W), x.shape
    nc = _get_nc()
    in_maps = [{"x": np.ascontiguousarray(x[b, 0], dtype=np.float32)} for b in range(N_CORES)]
    res = run_bass_kernel_spmd(nc, in_maps, core_ids=list(range(N_CORES)))
    return np.stack(
        [np.asarray(r["out"], dtype=np.float32) for r in res.results], axis=0
    )
